# revision 1
# baseline (speedup 1.0000x reference)
"""Trainium2 Bass kernel for nn_LocalitySelfAttention.

The module's attention scores get +1e9 added on the diagonal before the
softmax (torch's ``attn - diag(-1e9)``).  QK^T scores for randn inputs are
O(1), so every softmax row is an exact fp32 one-hot at the diagonal and
``attn @ v == v`` bit-exactly.  The whole module therefore reduces to

    out = x @ Wv.T @ w_proj.T + b_proj,      Wv = w_qkv[512:768]

which is a memory-bound GEMM.  The kernel shards the 8192 (B*N) rows across
the 8 NeuronCores (1024 rows each).  Each core:

  1. folds W2T[k,p] = sum_vd Wv[vd,k] * w_proj[p,vd] on the TensorEngine
     (4 matmuls contracting vd),
  2. broadcasts b_proj across partitions with a stride-0 DMA,
  3. computes out[n,p] = sum_k xT[k,n] * W2T[k,p] + b[p] as 8 PSUM tiles
     (2 matmuls each, K=128), bias added during the PSUM->SBUF copy.

The host only moves bytes: it flattens/transposes x (the TensorEngine
contracts along the partition axis, so x must arrive k-major) and
concatenates the per-core row blocks of the output.

DMA order matters: the small weight tensors are issued first so the fold
can start while the 1 MB x slice streams in (in chunks, so the main
matmuls pipeline behind the DMA).
"""

import os
import sys

import numpy as np

if "/opt/trn_rl_repo" not in sys.path:
    sys.path.insert(0, "/opt/trn_rl_repo")

B, N, C = 2, 4096, 256
ROWS = B * N              # 8192
NCORES = 8
RPC = ROWS // NCORES      # 1024 rows per core
NT = RPC // 128           # 8 row-tiles of 128 per core
XCHUNKS = 2               # xt DMA split (pipelines DMA with matmuls)

# matmul operand dtype: float32r streams 1 row/cycle (vs 4 for float32)
USE_F32R = os.environ.get("K_F32R", "0") == "1"
# PE warmup matmuls issued while input DMAs are in flight
NWARM = int(os.environ.get("K_NWARM", "12"))

_cache = {}


def _build():
    """Build + compile the per-core Bass program (same program, SPMD)."""
    import concourse.bacc as bacc
    import concourse.bass as bass
    import concourse.mybir as mybir
    import concourse.tile as tile

    f32 = mybir.dt.float32
    mm_dt = mybir.dt.float32r if USE_F32R else f32

    def mm(ap):
        return ap.bitcast(mm_dt) if USE_F32R else ap

    nc = bacc.Bacc(
        "TRN2",
        target_bir_lowering=False,
        debug=False,
        num_devices=NCORES,
    )

    xt_d = nc.dram_tensor("xt", [C, RPC], f32, kind="ExternalInput")
    wv_d = nc.dram_tensor("wv", [C, C], f32, kind="ExternalInput")
    wpt_d = nc.dram_tensor("wpt", [C, C], f32, kind="ExternalInput")
    b_d = nc.dram_tensor("b", [C], f32, kind="ExternalInput")
    out_d = nc.dram_tensor("out", [RPC, C], f32, kind="ExternalOutput")

    xt = xt_d.ap()
    wv = wv_d.ap()
    wpt = wpt_d.ap()
    b = b_d.ap()
    out = out_d.ap()

    with tile.TileContext(nc) as tc:
        with (
            tc.tile_pool(name="const", bufs=1) as cp,
            tc.tile_pool(name="io", bufs=4) as io,
            tc.tile_pool(name="psw", bufs=2, space="PSUM") as psw,
            tc.tile_pool(name="pso", bufs=4, space="PSUM") as pso,
        ):
            # ---- small loads first: weights + bias ----
            # Wv natural [vd, k] -> [p(vd), vdc, k]
            wv_sb = cp.tile([128, 2, C], f32)
            nc.sync.dma_start(out=wv_sb, in_=wv.rearrange("(vdc p) k -> p vdc k", p=128))

            # w_proj^T [vd, p] -> [p(vd), vdc, pcol]
            wpt_sb = cp.tile([128, 2, C], f32)
            nc.sync.dma_start(out=wpt_sb, in_=wpt.rearrange("(vdc p) q -> p vdc q", p=128))

            # bias broadcast across all 128 partitions (stride-0 partition DMA)
            bias_bc = cp.tile([128, C], f32)
            b_bcast = bass.AP(
                tensor=b.tensor,
                offset=b.offset,
                ap=[[0, 128]] + [list(d) for d in b.ap],
            )
            nc.gpsimd.dma_start(out=bias_bc, in_=b_bcast)

            # ---- x^T slice, k-major: [k=256, n=1024] -> [p, kc, n], chunked ----
            xt_sb = cp.tile([128, 2, RPC], f32)
            xt_v = xt.rearrange("(kc p) n -> p kc n", p=128)
            csz = RPC // XCHUNKS
            for ch in range(XCHUNKS):
                nc.sync.dma_start(
                    out=xt_sb[:, :, ch * csz:(ch + 1) * csz],
                    in_=xt_v[:, :, ch * csz:(ch + 1) * csz],
                )

            # ---- PE warmup: dummy matmuls during the input-DMA wait so the
            # HAM clock gate reaches 2.4 GHz before the real work ----
            warm_sb = cp.tile([128, 128], f32)
            nc.vector.memset(warm_sb, 0.0)
            warm_ps = psw.tile([128, 128], f32, tag="warm")
            for _ in range(NWARM):
                nc.tensor.matmul(warm_ps, warm_sb, warm_sb, start=True, stop=True)

            # ---- fold W2T[k, p] = sum_vd Wv[vd, k] * wpt[vd, p] ----
            w2t_sb = cp.tile([128, 2, C], f32)  # [p(k), kc, pcol]
            for kc in range(2):
                ps = psw.tile([128, C], f32)
                for vdc in range(2):
                    nc.tensor.matmul(
                        ps,
                        wv_sb[:, vdc, kc * 128:(kc + 1) * 128],
                        wpt_sb[:, vdc, :],
                        start=(vdc == 0),
                        stop=(vdc == 1),
                    )
                nc.vector.tensor_copy(w2t_sb[:, kc, :], ps)

            # ---- main GEMM: out[n, p] = sum_k xT[k, n] * W2T[k, p] + b[p] ----
            out_v = out.rearrange("(t p) m -> p t m", p=128)
            for t in range(NT):
                ps = pso.tile([128, C], f32)
                nc.tensor.matmul(
                    ps, mm(xt_sb[:, 0, t * 128:(t + 1) * 128]), mm(w2t_sb[:, 0, :]),
                    start=True, stop=False,
                )
                nc.tensor.matmul(
                    ps, mm(xt_sb[:, 1, t * 128:(t + 1) * 128]), mm(w2t_sb[:, 1, :]),
                    start=False, stop=True,
                )
                ot = io.tile([128, C], f32)
                nc.vector.tensor_add(ot, ps, bias_bc)
                nc.sync.dma_start(out=out_v[:, t, :], in_=ot)

    nc.compile()
    return nc


def run_sharded(inputs, trace=False, trace_cores=None):
    """Shard inputs, run on the 8 NeuronCores, gather.  Returns
    (full_output, BassKernelResults)."""
    from concourse.bass_utils import run_bass_kernel_spmd

    x = np.ascontiguousarray(np.asarray(inputs["x"], dtype=np.float32))
    w_qkv = np.ascontiguousarray(np.asarray(inputs["w_qkv"], dtype=np.float32))
    w_proj = np.ascontiguousarray(np.asarray(inputs["w_proj"], dtype=np.float32))
    b_proj = np.ascontiguousarray(np.asarray(inputs["b_proj"], dtype=np.float32))

    if "nc" not in _cache:
        _cache["nc"] = _build()
    nc = _cache["nc"]

    # host-side layout marshaling only (no FLOPs)
    xT = np.ascontiguousarray(x.reshape(ROWS, C).T)          # [256, 8192]
    wv = np.ascontiguousarray(w_qkv[2 * C:3 * C])            # [256, 256]
    wpt = np.ascontiguousarray(w_proj.T)                     # [256, 256]

    in_maps = [
        {
            "xt": np.ascontiguousarray(xT[:, c * RPC:(c + 1) * RPC]),
            "wv": wv,
            "wpt": wpt,
            "b": b_proj,
        }
        for c in range(NCORES)
    ]

    res = run_bass_kernel_spmd(
        nc,
        in_maps,
        core_ids=list(range(NCORES)),
        trace=trace,
        trace_cores=trace_cores,
    )
    out = np.concatenate(
        [res.results[c]["out"] for c in range(NCORES)], axis=0
    )  # [8192, 256]
    return out.reshape(B, N, C), res


def kernel(x, w_qkv, w_proj, b_proj, temperature):
    out, _ = run_sharded(
        {"x": x, "w_qkv": w_qkv, "w_proj": w_proj, "b_proj": b_proj}
    )
    return out



# revision 6
# speedup vs baseline: 1.0335x; 1.0335x over previous
"""Trainium2 Bass kernel for nn_LocalitySelfAttention.

The module's attention scores get +1e9 added on the diagonal before the
softmax (torch's ``attn - diag(-1e9)``).  QK^T scores for randn inputs are
O(1), so every softmax row is an exact fp32 one-hot at the diagonal and
``attn @ v == v`` bit-exactly.  The whole module therefore reduces to

    out = x @ Wv.T @ w_proj.T + b_proj,      Wv = w_qkv[512:768]

which is a memory-bound GEMM.  The kernel shards the 8192 (B*N) rows across
the 8 NeuronCores (1024 rows each).  Each core:

  1. folds W2T[k,p] = sum_vd Wv[vd,k] * w_proj[p,vd] on the TensorEngine
     (4 matmuls contracting vd),
  2. broadcasts b_proj across partitions with a stride-0 DMA,
  3. computes out[n,p] = sum_k xT[k,n] * W2T[k,p] + b[p] as 8 PSUM tiles
     (2 matmuls each, K=128), bias added during the PSUM->SBUF copy.

The host only moves bytes: it flattens/transposes x (the TensorEngine
contracts along the partition axis, so x must arrive k-major) and
concatenates the per-core row blocks of the output.

DMA order matters: the small weight tensors are issued first so the fold
can start while the 1 MB x slice streams in (in chunks, so the main
matmuls pipeline behind the DMA).
"""

import os
import sys

import numpy as np

if "/opt/trn_rl_repo" not in sys.path:
    sys.path.insert(0, "/opt/trn_rl_repo")

B, N, C = 2, 4096, 256
ROWS = B * N              # 8192
NCORES = 8
RPC = ROWS // NCORES      # 1024 rows per core
NT = RPC // 128           # 8 row-tiles of 128 per core
XCHUNKS = 2               # xt DMA split (pipelines DMA with matmuls)

# matmul operand dtype: float32r streams 1 row/cycle (vs 4 for float32)
USE_F32R = os.environ.get("K_F32R", "0") == "1"
# PE warmup matmuls issued while input DMAs are in flight
NWARM = int(os.environ.get("K_NWARM", "12"))

_cache = {}


def _build():
    """Build + compile the per-core Bass program (same program, SPMD)."""
    import concourse.bacc as bacc
    import concourse.bass as bass
    import concourse.mybir as mybir
    import concourse.tile as tile

    f32 = mybir.dt.float32
    mm_dt = mybir.dt.float32r if USE_F32R else f32

    def mm(ap):
        return ap.bitcast(mm_dt) if USE_F32R else ap

    nc = bacc.Bacc(
        "TRN2",
        target_bir_lowering=False,
        debug=False,
        num_devices=NCORES,
    )

    # xt is typed f32r end-to-end (DRAM + SBUF) so the BIR verifier sees a
    # consistently-rounded producer chain for the f32r matmuls; the bytes
    # are plain fp32 either way.
    xt_d = nc.dram_tensor("xt", [C, RPC], mm_dt, kind="ExternalInput")
    wv_d = nc.dram_tensor("wv", [C, C], f32, kind="ExternalInput")
    wpt_d = nc.dram_tensor("wpt", [C, C], f32, kind="ExternalInput")
    b_d = nc.dram_tensor("b", [C], f32, kind="ExternalInput")
    out_d = nc.dram_tensor("out", [RPC, C], f32, kind="ExternalOutput")

    xt = xt_d.ap()
    wv = wv_d.ap()
    wpt = wpt_d.ap()
    b = b_d.ap()
    out = out_d.ap()

    with tile.TileContext(nc) as tc:
        with (
            tc.tile_pool(name="const", bufs=1) as cp,
            tc.tile_pool(name="io", bufs=4) as io,
            tc.tile_pool(name="psw", bufs=2, space="PSUM") as psw,
            tc.tile_pool(name="pso", bufs=4, space="PSUM") as pso,
        ):
            # ---- small loads first: weights + bias ----
            # Wv natural [vd, k] -> [p(vd), vdc, k]
            wv_sb = cp.tile([128, 2, C], f32)
            nc.sync.dma_start(out=wv_sb, in_=wv.rearrange("(vdc p) k -> p vdc k", p=128))

            # w_proj^T [vd, p] -> [p(vd), vdc, pcol]
            wpt_sb = cp.tile([128, 2, C], f32)
            nc.sync.dma_start(out=wpt_sb, in_=wpt.rearrange("(vdc p) q -> p vdc q", p=128))

            # bias broadcast across all 128 partitions (stride-0 partition DMA)
            bias_bc = cp.tile([128, C], f32)
            b_bcast = bass.AP(
                tensor=b.tensor,
                offset=b.offset,
                ap=[[0, 128]] + [list(d) for d in b.ap],
            )
            nc.gpsimd.dma_start(out=bias_bc, in_=b_bcast)

            # ---- x^T slice, k-major: [k=256, n=1024] -> [p, kc, n], chunked ----
            xt_sb = cp.tile([128, 2, RPC], mm_dt)
            xt_v = xt.rearrange("(kc p) n -> p kc n", p=128)
            csz = RPC // XCHUNKS
            for ch in range(XCHUNKS):
                nc.sync.dma_start(
                    out=xt_sb[:, :, ch * csz:(ch + 1) * csz],
                    in_=xt_v[:, :, ch * csz:(ch + 1) * csz],
                )

            # ---- PE warmup: dummy matmuls during the input-DMA wait so the
            # HAM clock gate reaches 2.4 GHz before the real work ----
            warm_sb = cp.tile([128, 128], f32)
            nc.vector.memset(warm_sb, 0.0)
            warm_ps = psw.tile([128, 128], f32, tag="warm")
            for _ in range(NWARM):
                nc.tensor.matmul(warm_ps, warm_sb, warm_sb, start=True, stop=True)

            # ---- fold W2T[k, p] = sum_vd Wv[vd, k] * wpt[vd, p] ----
            # w2t is consumed by the f32r matmuls below, so the PSUM->SBUF
            # copy must emit f32r (the BIR verifier rejects an f32-typed
            # producer feeding an FP32r matmult).
            w2t_sb = cp.tile([128, 2, C], mm_dt)  # [p(k), kc, pcol]
            for kc in range(2):
                ps = psw.tile([128, C], f32)
                for vdc in range(2):
                    nc.tensor.matmul(
                        ps,
                        wv_sb[:, vdc, kc * 128:(kc + 1) * 128],
                        wpt_sb[:, vdc, :],
                        start=(vdc == 0),
                        stop=(vdc == 1),
                    )
                nc.vector.tensor_copy(w2t_sb[:, kc, :], ps)

            # ---- main GEMM: out[n, p] = sum_k xT[k, n] * W2T[k, p] + b[p] ----
            out_v = out.rearrange("(t p) m -> p t m", p=128)
            for t in range(NT):
                ps = pso.tile([128, C], f32)
                nc.tensor.matmul(
                    ps, xt_sb[:, 0, t * 128:(t + 1) * 128], w2t_sb[:, 0, :],
                    start=True, stop=False,
                )
                nc.tensor.matmul(
                    ps, xt_sb[:, 1, t * 128:(t + 1) * 128], w2t_sb[:, 1, :],
                    start=False, stop=True,
                )
                ot = io.tile([128, C], f32)
                nc.vector.tensor_add(ot, ps, bias_bc)
                nc.sync.dma_start(out=out_v[:, t, :], in_=ot)

    nc.compile()
    return nc


def run_sharded(inputs, trace=False, trace_cores=None):
    """Shard inputs, run on the 8 NeuronCores, gather.  Returns
    (full_output, BassKernelResults)."""
    from concourse.bass_utils import run_bass_kernel_spmd

    x = np.ascontiguousarray(np.asarray(inputs["x"], dtype=np.float32))
    w_qkv = np.ascontiguousarray(np.asarray(inputs["w_qkv"], dtype=np.float32))
    w_proj = np.ascontiguousarray(np.asarray(inputs["w_proj"], dtype=np.float32))
    b_proj = np.ascontiguousarray(np.asarray(inputs["b_proj"], dtype=np.float32))

    if "nc" not in _cache:
        _cache["nc"] = _build()
    nc = _cache["nc"]

    # host-side layout marshaling only (no FLOPs)
    xT = np.ascontiguousarray(x.reshape(ROWS, C).T)          # [256, 8192]
    wv = np.ascontiguousarray(w_qkv[2 * C:3 * C])            # [256, 256]
    wpt = np.ascontiguousarray(w_proj.T)                     # [256, 256]

    in_maps = [
        {
            "xt": np.ascontiguousarray(xT[:, c * RPC:(c + 1) * RPC]),
            "wv": wv,
            "wpt": wpt,
            "b": b_proj,
        }
        for c in range(NCORES)
    ]

    res = run_bass_kernel_spmd(
        nc,
        in_maps,
        core_ids=list(range(NCORES)),
        trace=trace,
        trace_cores=trace_cores,
    )
    out = np.concatenate(
        [res.results[c]["out"] for c in range(NCORES)], axis=0
    )  # [8192, 256]
    return out.reshape(B, N, C), res


def kernel(x, w_qkv, w_proj, b_proj, temperature):
    out, _ = run_sharded(
        {"x": x, "w_qkv": w_qkv, "w_proj": w_proj, "b_proj": b_proj}
    )
    return out

